# revision 1
# baseline (speedup 1.0000x reference)
"""Trainium2 Bass kernel for ConditionedSparseAttention.

Problem: B=2, T_IN=2048, T_COND=1024 (S=3072), D=1024, H=16, HD=64, W=512.
The window mask depends only on end_inds[b], NOT the query position: every
query attends to exactly the same 1024 keys (rows [e-W, e) of each of the two
segments, since end_inds in [W, 1024)).  So attention is a softmax over a
fixed 1024-key set and K/V projections are only needed for those 1024 rows.

Sharding: 8 cores = 2 batches x 4 query-shards of 768 queries.  Each core:
  Q^T proj (its 768 queries), then per head-pair c: K^T proj chunk c ->
  scores^T [k, q] -> exp (ScalarE) -> attn@V with ones-augmented V
  (denominator for free) -> normalize via ones-matmul partition broadcast;
  V projection (in [k, vdim] layout, quartered over vdim) is interleaved so
  attn@V for pair c only needs V-quarter c//2.  Finally the output
  projection in transposed form (bias lands on partitions).

All matmul operands are float32r (TF32-like: full PE rate at free-dim >=256,
~2^-13 relative precision).  Scores are small (|s| < 4 for these inputs), so
softmax needs no max subtraction and exp cannot overflow.
"""
import os
import sys
import tempfile

# The libneuronxla compile cache keys on an HLO hash that does NOT cover the
# embedded BIR payload, so a stale NEFF from a previous kernel revision can be
# silently reused.  Pin the cache to a fresh per-process dir so the compiled
# NEFF always matches this code.
os.environ["NEURON_COMPILE_CACHE_URL"] = tempfile.mkdtemp(prefix="bass_kernel_cache_")

try:
    import concourse  # noqa: F401
except ImportError:
    sys.path.insert(0, "/opt/trn_rl_repo")

import numpy as np

import concourse.bacc as bacc
import concourse.tile as tile
import concourse.mybir as mybir
from concourse.bass_utils import run_bass_kernel_spmd

# ---- problem constants (hardcoded per harness contract) ----
B, T_IN, T_COND, D, H, HD, W = 2, 2048, 1024, 1024, 16, 64, 512
S = T_IN + T_COND            # 3072
NQSH = 4                     # query shards per batch
SQ = S // NQSH               # 768 queries per core
NCH = D // 128               # 8 d-chunks
KT = 1024 // 128             # 8 k-tiles of selected keys
QCS = [(0, 384), (384, 384)]  # q sub-chunks (fp32r wants N>=256)
SEL = 2 * W                  # 1024 selected keys

F32 = mybir.dt.float32
F32R = mybir.dt.float32r
AF = mybir.ActivationFunctionType
ALU = mybir.AluOpType

_CACHE = {}


def _build():
    if "nc" in _CACHE:
        return _CACHE["nc"]

    nc = bacc.Bacc("TRN2", target_bir_lowering=False, debug=False,
                   enable_asserts=True, num_devices=8)

    xq_d = nc.dram_tensor("xq", (128, NCH, SQ), F32R, kind="ExternalInput").ap()
    xkv_d = nc.dram_tensor("xkv", (128, NCH, SEL), F32R, kind="ExternalInput").ap()
    wqkv_d = nc.dram_tensor("wqkv", (128, NCH, 3 * D), F32R, kind="ExternalInput").ap()
    wo_d = nc.dram_tensor("wo", (128, NCH, D), F32R, kind="ExternalInput").ap()
    bq_d = nc.dram_tensor("bq", (128, NCH), F32, kind="ExternalInput").ap()
    bk_d = nc.dram_tensor("bk", (128, NCH), F32, kind="ExternalInput").ap()
    bo_d = nc.dram_tensor("bo", (128, NCH), F32, kind="ExternalInput").ap()
    y_d = nc.dram_tensor("y", (128, NCH, SQ), F32, kind="ExternalOutput").ap()

    with tile.TileContext(nc) as tc:
        with (
            tc.tile_pool(name="const", bufs=1) as cpool,
            tc.tile_pool(name="xin", bufs=2) as xin_pool,
            tc.tile_pool(name="wstr", bufs=2) as wpool,
            tc.tile_pool(name="wv", bufs=2) as wvpool,
            tc.tile_pool(name="work", bufs=1) as work,
            tc.tile_pool(name="exps", bufs=8) as epool,
            tc.tile_pool(name="stage", bufs=2) as spool,
            tc.tile_pool(name="ps", bufs=2, space="PSUM") as ps,        # proj [128,512] x2
            tc.tile_pool(name="ps_s", bufs=2, space="PSUM") as ps_s,    # S [128,2,512] x2
            tc.tile_pool(name="ps_o", bufs=1, space="PSUM") as ps_o,    # o [128,512]
            tc.tile_pool(name="ps_b", bufs=1, space="PSUM") as ps_b,    # b [64,512]
        ):
            # ---- query slab halves (DMA first) ----
            xq_a = xin_pool.tile([128, NCH, 512], F32R, tag="xin", name="xq_a")
            xq_b = xin_pool.tile([128, NCH, 512], F32R, tag="xin", name="xq_b")
            nc.sync.dma_start(xq_a[:, :, 0:256], xq_d[:, :, 0:256])
            nc.sync.dma_start(xq_a[:, :, 256:512], xq_d[:, :, 256:512])
            nc.gpsimd.dma_start(xq_b[:, :, 0:SQ - 512], xq_d[:, :, 512:SQ])

            # ---- small constants ----
            bq_sb = cpool.tile([128, NCH], F32, tag="bq")
            bk_sb = cpool.tile([128, NCH], F32, tag="bk")
            bo_sb = cpool.tile([128, NCH], F32, tag="bo")
            nc.sync.dma_start(bq_sb[:], bq_d[:])
            nc.sync.dma_start(bk_sb[:], bk_d[:])
            nc.sync.dma_start(bo_sb[:], bo_d[:])

            ones_f = cpool.tile([128, HD], F32, tag="ones_f")
            nc.vector.memset(ones_f[:], 1.0)
            ones_r = cpool.tile([1, HD], F32R, tag="ones_r")
            nc.vector.tensor_copy(ones_r[:], ones_f[0:1, :])

            # ---- persistent per-chunk tensors ----
            q_t = [work.tile([128, SQ], F32R, tag=f"qt{c}", name=f"qt{c}")
                   for c in range(NCH)]
            k_t = [work.tile([128, SEL], F32R, tag=f"kt{c}", name=f"kt{c}")
                   for c in range(NCH)]
            v_aug = [work.tile([128, H, HD + 1], F32R, tag=f"va{c}", name=f"va{c}")
                     for c in range(KT)]
            o_all = [work.tile([128, SQ], F32R, tag=f"oa{c}", name=f"oa{c}")
                     for c in range(NCH)]

            # ones column of V_aug (region-disjoint from the V data writes)
            for st in range(KT):
                nc.vector.tensor_copy(
                    v_aug[st][:, :, HD:HD + 1],
                    ones_f[:, 0:H].rearrange("p (h one) -> p h one", one=1))

            # prefetch first two V-weight quarters (vdim 0:256, 256:512)
            wv_q = [wvpool.tile([128, NCH, 256], F32R, tag="wv", name=f"wv{q}")
                    for q in range(4)]
            nc.gpsimd.dma_start(wv_q[0][:], wqkv_d[:, :, 2 * D:2 * D + 256])
            nc.gpsimd.dma_start(wv_q[1][:], wqkv_d[:, :, 2 * D + 256:2 * D + 512])

            # ---- phase A: Q^T projection: q_t = 0.125*(Wq^T.T @ xq^T + b) ----
            for dt in range(NCH):
                w_dt = wpool.tile([128, NCH, 128], F32R, tag="w")
                nc.sync.dma_start(w_dt[:], wqkv_d[:, :, dt * 128:(dt + 1) * 128])
                for src, lo, xo, n in ((xq_a, 0, 0, 256), (xq_a, 256, 256, 256),
                                       (xq_b, 512, 0, 256)):
                    psq = ps.tile([128, 512], F32, tag="proj")
                    for dc in range(NCH):
                        nc.tensor.matmul(
                            psq[:, 0:n], w_dt[:, dc, :], src[:, dc, xo:xo + n],
                            start=(dc == 0), stop=(dc == NCH - 1))
                    nc.vector.tensor_scalar(
                        q_t[dt][:, lo:lo + n], psq[:, 0:n],
                        bq_sb[:, dt:dt + 1], 0.125, ALU.add, ALU.mult)

            # ---- selected-rows slab halves (reuse the xq slots) ----
            xkv_h = [xin_pool.tile([128, NCH, 512], F32R, tag="xin", name=f"xkv{i}")
                     for i in range(2)]
            for i in range(2):
                nc.gpsimd.dma_start(xkv_h[i][:, :, 0:256], xkv_d[:, :, 512 * i:512 * i + 256])
                nc.gpsimd.dma_start(xkv_h[i][:, :, 256:512], xkv_d[:, :, 512 * i + 256:512 * i + 512])

            # ---- main loop: K proj chunk + V quarter + attention, per head-pair ----
            for c in range(NCH):
                # V projection quarter (vdim c//2): produces heads 4*(c//2)..+3
                if c % 2 == 0:
                    vq = c // 2
                    if vq >= 2:
                        nc.gpsimd.dma_start(
                            wv_q[vq][:],
                            wqkv_d[:, :, 2 * D + 256 * vq:2 * D + 256 * (vq + 1)])
                    for st in range(KT):
                        psv = ps.tile([128, 512], F32, tag="proj")
                        xh = xkv_h[st // 4]
                        so = (st % 4) * 128
                        for dc in range(NCH):
                            nc.tensor.matmul(
                                psv[:, 0:256], xh[:, dc, so:so + 128],
                                wv_q[vq][:, dc, :],
                                start=(dc == 0), stop=(dc == NCH - 1))
                        nc.vector.tensor_copy(
                            v_aug[st][:, 4 * vq:4 * (vq + 1), 0:HD],
                            psv[:, 0:256].rearrange("p (h hd) -> p h hd", h=4))

                # K^T projection for chunk c (heads 2c, 2c+1)
                w_dt = wpool.tile([128, NCH, 128], F32R, tag="w")
                nc.sync.dma_start(w_dt[:], wqkv_d[:, :, D + c * 128:D + (c + 1) * 128])
                for hi, xo in ((0, 0), (0, 256), (1, 0), (1, 256)):
                    lo = 512 * hi + xo
                    psk = ps.tile([128, 512], F32, tag="proj")
                    for dc in range(NCH):
                        nc.tensor.matmul(
                            psk[:, 0:256], w_dt[:, dc, :],
                            xkv_h[hi][:, dc, xo:xo + 256],
                            start=(dc == 0), stop=(dc == NCH - 1))
                    nc.vector.tensor_scalar(
                        k_t[c][:, lo:lo + 256], psk[:, 0:256],
                        bk_sb[:, c:c + 1], None, ALU.add)

                # attention for the two heads of this pair, scores row-packed:
                # the two K=64 matmuls target PE row strips 0/64 and overlap
                for (q0, qn) in QCS:
                    # exp output in per-(head, 2-kt-group) subtiles so attn@V
                    # releases them incrementally for the next chunk's exps
                    exg = [[None] * (KT // 2) for _ in range(2)]
                    for g in range(KT // 2):
                        s_pair = [ps_s.tile([128, 2, 512], F32, tag="S",
                                            name=f"s{c}_{g}_{hf}")
                                  for hf in range(2)]
                        for hf in range(2):
                            exg[hf][g] = epool.tile([128, 2, 384], F32R, tag="expT",
                                                    name=f"ex{c}_{q0}_{hf}_{g}")
                        for j in range(2):
                            kt_i = 2 * g + j
                            for half in range(2):
                                pb = 64 * half
                                nc.tensor.matmul(
                                    s_pair[half][:, j, 0:qn],
                                    k_t[c][pb:pb + HD, kt_i * 128:(kt_i + 1) * 128],
                                    q_t[c][pb:pb + HD, q0:q0 + qn],
                                    start=True, stop=True, tile_position=(pb, 0))
                        for half in range(2):
                            nc.scalar.activation(
                                exg[half][g][:, :, 0:qn],
                                s_pair[half][:, :, 0:qn], AF.Exp)
                    for half in range(2):
                        h = 2 * c + half
                        pb = 64 * half
                        o_ps = ps_o.tile([128, 512], F32, tag="o")
                        for kc in range(KT):
                            nc.tensor.matmul(
                                o_ps[0:HD + 1, 0:qn],
                                v_aug[kc][:, h, :],
                                exg[half][kc // 2][:, kc % 2, 0:qn],
                                start=(kc == 0), stop=(kc == KT - 1))
                        rec = spool.tile([1, 512], F32R, tag="rec")
                        with nc.allow_low_precision(reason="softmax recip feeds fp32r bcast"):
                            nc.vector.reciprocal(rec[:, 0:qn], o_ps[HD:HD + 1, 0:qn])
                        b_ps = ps_b.tile([64, 512], F32, tag="b")
                        nc.tensor.matmul(b_ps[:, 0:qn], ones_r[:], rec[:, 0:qn],
                                         start=True, stop=True)
                        bc_sb = spool.tile([64, 512], F32, tag="bc")
                        nc.vector.tensor_copy(bc_sb[:, 0:qn], b_ps[:, 0:qn])
                        nc.vector.tensor_tensor(
                            o_all[c][pb:pb + HD, q0:q0 + qn], o_ps[0:HD, 0:qn],
                            bc_sb[:, 0:qn], ALU.mult)

            # ---- phase D: output projection, transposed: out^T = Wo^T.T @ O^T ----
            for dt in range(NCH):
                w_dt = wpool.tile([128, NCH, 128], F32R, tag="w")
                nc.sync.dma_start(w_dt[:], wo_d[:, :, dt * 128:(dt + 1) * 128])
                y_sb = spool.tile([128, SQ], F32, tag="ysb")
                for lo, n in ((0, 512), (512, 256)):
                    psf = ps.tile([128, 512], F32, tag="proj")
                    for dc in range(NCH):
                        nc.tensor.matmul(
                            psf[:, 0:n], w_dt[:, dc, :], o_all[dc][:, lo:lo + n],
                            start=(dc == 0), stop=(dc == NCH - 1))
                    nc.vector.tensor_scalar(
                        y_sb[:, lo:lo + n], psf[:, 0:n],
                        bo_sb[:, dt:dt + 1], None, ALU.add)
                nc.sync.dma_start(y_d[:, dt, :], y_sb[:])

    nc.compile()
    _CACHE["nc"] = nc
    return nc


def _to_pko(a2d):
    """(D_in, M) row-major -> [128, D_in//128, M] with d = ko*128 + p."""
    d_in, m = a2d.shape
    return np.ascontiguousarray(
        a2d.reshape(d_in // 128, 128, m).transpose(1, 0, 2))


def kernel(x, condition, end_inds, in_proj_w, in_proj_b, out_w, out_b):
    nc = _build()

    x = np.asarray(x, dtype=np.float32)
    condition = np.asarray(condition, dtype=np.float32)
    end_inds = np.asarray(end_inds, dtype=np.int32)
    in_proj_w = np.asarray(in_proj_w, dtype=np.float32)
    in_proj_b = np.asarray(in_proj_b, dtype=np.float32)
    out_w = np.asarray(out_w, dtype=np.float32)
    out_b = np.asarray(out_b, dtype=np.float32)

    # shared across cores
    wqkv_t = _to_pko(np.ascontiguousarray(in_proj_w.T))           # [128, 8, 3072]
    wo_t = _to_pko(np.ascontiguousarray(out_w.T))                 # [128, 8, 1024]
    bq = np.ascontiguousarray((0.125 * in_proj_b[:D]).reshape(NCH, 128).T)
    bk = np.ascontiguousarray(in_proj_b[D:2 * D].reshape(NCH, 128).T)
    bo_eff = out_b + out_w @ in_proj_b[2 * D:3 * D]
    bo = np.ascontiguousarray(bo_eff.astype(np.float32).reshape(NCH, 128).T)

    in_maps = []
    for core in range(8):
        b, qs = divmod(core, NQSH)
        inp = np.concatenate([x[b], condition[b]], axis=0)        # (3072, 1024)
        e = int(end_inds[b])
        sel = np.concatenate([inp[e - W:e], inp[T_IN + e - W:T_IN + e]], axis=0)
        xq_t = _to_pko(np.ascontiguousarray(inp[qs * SQ:(qs + 1) * SQ].T))
        xkv_t = _to_pko(np.ascontiguousarray(sel.T))
        in_maps.append({
            "xq": xq_t, "xkv": xkv_t, "wqkv": wqkv_t, "wo": wo_t,
            "bq": bq, "bk": bk, "bo": bo,
        })

    res = run_bass_kernel_spmd(nc, in_maps, core_ids=list(range(8)))

    out = np.empty((B, S, D), dtype=np.float32)
    for core in range(8):
        b, qs = divmod(core, NQSH)
        yv = res.results[core]["y"]                               # [128, 8, 768]
        slab = yv.transpose(1, 0, 2).reshape(D, SQ).T             # (768, 1024)
        out[b, qs * SQ:(qs + 1) * SQ] = slab
    return out



# revision 2
# speedup vs baseline: 1.2603x; 1.2603x over previous
"""Trainium2 Bass kernel for ConditionedSparseAttention — head-sharded v2.

Problem: B=2, T_IN=2048, T_COND=1024 (S=3072), D=1024, H=16, HD=64, W=512.
The window mask depends only on end_inds[b]: every query attends to the same
1024 keys (rows [e-W, e) of each segment).  Attention reduces to a softmax
over a fixed 1024-key set; K/V projections are needed only for those rows.

Sharding: 8 cores = 2 batches x 4 head-quarters (4 heads / 256 dims each).
Each core computes, for its 4 heads: Q^T projection (all 3072 queries),
K^T/V projections (1024 selected keys; NO cross-core duplication), scores^T
[key, q] -> exp (ScalarE, bf16) -> flipped attn@V out[q, hd+1] with a
ones-augmented V column giving the softmax denominator per query on the
PSUM partition axis -> per-partition normalize (VectorE) -> PE transpose
back to [od, q] -> output-projection PARTIAL y_part = Wo[:, od_mine] @ o^T.
The host sums the 4 partial y's per batch and adds the folded output bias.

All matmuls run in bf16 (full PE rate at any free size); PSUM accumulates
in fp32.  Scores are small (|s| < 4), so softmax needs no max subtraction.
"""
import os
import sys
import tempfile

# The libneuronxla compile cache keys on an HLO hash that does NOT cover the
# embedded BIR payload; pin the cache to a fresh per-process dir so the
# compiled NEFF always matches this code.
os.environ["NEURON_COMPILE_CACHE_URL"] = tempfile.mkdtemp(prefix="bass_kernel_cache_")

try:
    import concourse  # noqa: F401
except ImportError:
    sys.path.insert(0, "/opt/trn_rl_repo")

import numpy as np
import ml_dtypes

import concourse.bacc as bacc
import concourse.tile as tile
import concourse.mybir as mybir
from concourse.bass_utils import run_bass_kernel_spmd

# ---- problem constants (hardcoded per harness contract) ----
B, T_IN, T_COND, D, H, HD, W = 2, 2048, 1024, 1024, 16, 64, 512
S = T_IN + T_COND            # 3072
SEL = 2 * W                  # 1024 selected keys
NCH = D // 128               # 8 input-dim chunks
HPC = 4                      # heads per core
ODC = HPC * HD               # 256 o-dims per core (2 chunks of 128)
QB = 512                     # query block
NQB = S // QB                # 6
KT = SEL // 128              # 8 key tiles

F32 = mybir.dt.float32
BF16 = mybir.dt.bfloat16
AF = mybir.ActivationFunctionType
ALU = mybir.AluOpType

_CACHE = {}


def _build():
    if "nc" in _CACHE:
        return _CACHE["nc"]

    nc = bacc.Bacc("TRN2", target_bir_lowering=False, debug=False,
                   enable_asserts=True, num_devices=8)

    x_d = nc.dram_tensor("x", (128, NCH, S), BF16, kind="ExternalInput").ap()
    xkv_d = nc.dram_tensor("xkv", (128, NCH, SEL), BF16, kind="ExternalInput").ap()
    wq_d = nc.dram_tensor("wq", (128, NCH, ODC), BF16, kind="ExternalInput").ap()
    wk_d = nc.dram_tensor("wk", (128, NCH, ODC), BF16, kind="ExternalInput").ap()
    wv_d = nc.dram_tensor("wv", (128, NCH, ODC), BF16, kind="ExternalInput").ap()
    wo_d = nc.dram_tensor("wo", (128, 2, D), BF16, kind="ExternalInput").ap()
    bq_d = nc.dram_tensor("bq", (128, 2), F32, kind="ExternalInput").ap()
    bk_d = nc.dram_tensor("bk", (128, 2), F32, kind="ExternalInput").ap()
    id_d = nc.dram_tensor("ident", (128, 128), BF16, kind="ExternalInput").ap()
    y_d = nc.dram_tensor("y", (128, NCH, S), F32, kind="ExternalOutput").ap()

    with tile.TileContext(nc) as tc:
        with (
            tc.tile_pool(name="const", bufs=1) as cpool,
            tc.tile_pool(name="wts", bufs=1) as wpool,
            tc.tile_pool(name="xin", bufs=1) as xpool,
            tc.tile_pool(name="work", bufs=1) as work,
            tc.tile_pool(name="exps", bufs=2) as epool,
            tc.tile_pool(name="osb", bufs=2) as opool,
            tc.tile_pool(name="recs", bufs=2) as rpool,
            tc.tile_pool(name="ysb", bufs=3) as ypool,
            tc.tile_pool(name="ps", bufs=2, space="PSUM") as ps,       # proj f32 x2 + tp bf16 x1
            tc.tile_pool(name="ps_s", bufs=2, space="PSUM") as ps_s,   # scores [128,2,512] x2
            tc.tile_pool(name="ps_o", bufs=1, space="PSUM") as ps_o,   # attn@V [128,4,65]
        ):
            # ---- small constants ----
            bq_sb = cpool.tile([128, 2], F32, tag="bq")
            bk_sb = cpool.tile([128, 2], F32, tag="bk")
            id_sb = cpool.tile([128, 128], BF16, tag="ident")
            nc.sync.dma_start(bq_sb[:], bq_d[:])
            nc.sync.dma_start(bk_sb[:], bk_d[:])
            nc.sync.dma_start(id_sb[:], id_d[:])

            # ---- weights / inputs DMA ----
            wk_sb = wpool.tile([128, NCH, ODC], BF16, tag="wk")
            wv_sb = wpool.tile([128, NCH, ODC], BF16, tag="wv")
            wq_sb = wpool.tile([128, NCH, ODC], BF16, tag="wq")
            wo_sb = wpool.tile([128, 2, D], BF16, tag="wo")
            xkv_sb = xpool.tile([128, NCH, SEL], BF16, tag="xkv")
            x_sb = xpool.tile([128, NCH, S], BF16, tag="x")

            nc.sync.dma_start(wk_sb[:], wk_d[:])
            nc.sync.dma_start(xkv_sb[:, :, 0:512], xkv_d[:, :, 0:512])
            nc.gpsimd.dma_start(xkv_sb[:, :, 512:1024], xkv_d[:, :, 512:1024])
            nc.gpsimd.dma_start(wv_sb[:], wv_d[:])
            nc.sync.dma_start(wq_sb[:], wq_d[:])
            nc.sync.dma_start(wo_sb[:], wo_d[:])
            for qb in range(NQB):
                eng = nc.gpsimd if qb % 2 == 0 else nc.sync
                eng.dma_start(x_sb[:, :, qb * QB:(qb + 1) * QB],
                              x_d[:, :, qb * QB:(qb + 1) * QB])

            # ---- persistent tensors ----
            q_t = work.tile([128, 2, S], BF16, tag="qt")
            k_t = work.tile([128, 2, SEL], BF16, tag="kt")
            o_all = work.tile([128, 2, S], BF16, tag="oall")
            v_aug = [work.tile([128, HPC, HD + 1], BF16, tag=f"va{kt}",
                               name=f"va{kt}") for kt in range(KT)]

            # ---- K^T projection: k_t[kdim, key] ----
            for dt in range(2):
                for nb in range(2):
                    psk = ps.tile([128, QB], F32, tag="proj", name=f"psk{dt}{nb}")
                    for dc in range(NCH):
                        nc.tensor.matmul(
                            psk[:], wk_sb[:, dc, dt * 128:(dt + 1) * 128],
                            xkv_sb[:, dc, nb * QB:(nb + 1) * QB],
                            start=(dc == 0), stop=(dc == NCH - 1))
                    nc.vector.tensor_scalar(
                        k_t[:, dt, nb * QB:(nb + 1) * QB], psk[:],
                        bk_sb[:, dt:dt + 1], None, ALU.add)

            # ---- V projection (ones-augmented): v_aug[key, h, vd] ----
            for kt in range(KT):
                psv = ps.tile([128, QB], F32, tag="proj", name=f"psv{kt}")
                for dc in range(NCH):
                    nc.tensor.matmul(
                        psv[:, 0:ODC], xkv_sb[:, dc, kt * 128:(kt + 1) * 128],
                        wv_sb[:, dc, :], start=(dc == 0), stop=(dc == NCH - 1))
                nc.vector.tensor_copy(
                    v_aug[kt][:, :, 0:HD],
                    psv[:, 0:ODC].rearrange("p (h d) -> p h d", h=HPC))
                nc.vector.memset(v_aug[kt][:, :, HD:HD + 1], 1.0)

            # ---- Q^T projection for one query block ----
            def qproj(qb):
                for dt in range(2):
                    psq = ps.tile([128, QB], F32, tag="proj", name=f"psq{dt}_{qb}")
                    for dc in range(NCH):
                        nc.tensor.matmul(
                            psq[:], wq_sb[:, dc, dt * 128:(dt + 1) * 128],
                            x_sb[:, dc, qb * QB:(qb + 1) * QB],
                            start=(dc == 0), stop=(dc == NCH - 1))
                    nc.vector.tensor_scalar(
                        q_t[:, dt, qb * QB:(qb + 1) * QB], psq[:],
                        bq_sb[:, dt:dt + 1], None, ALU.add)

            qproj(0)
            qproj(1)

            # ---- main loop: attention + output projection per query block ----
            for qb in range(NQB):
                o_sb = opool.tile([128, 4, ODC], BF16, tag="osb", name=f"osb{qb}")
                for h in range(HPC):
                    pb = 64 * (h % 2)
                    ch = h // 2
                    exp_t = epool.tile([128, KT, QB], BF16, tag="exp",
                                       name=f"exp{qb}_{h}")
                    for g in range(4):
                        s_ps = ps_s.tile([128, 2, QB], F32, tag="S",
                                         name=f"s{qb}_{h}_{g}")
                        for j in range(2):
                            kt = 2 * g + j
                            nc.tensor.matmul(
                                s_ps[:, j, :],
                                k_t[pb:pb + HD, ch, kt * 128:(kt + 1) * 128],
                                q_t[pb:pb + HD, ch, qb * QB:(qb + 1) * QB],
                                start=True, stop=True, tile_position=(pb, 0))
                        nc.scalar.activation(
                            exp_t[:, 2 * g:2 * g + 2, :], s_ps[:], AF.Exp)
                    o_ps = ps_o.tile([128, 4, HD + 1], F32, tag="o",
                                     name=f"o{qb}_{h}")
                    for qt in range(4):
                        for kt in range(KT):
                            nc.tensor.matmul(
                                o_ps[:, qt, :],
                                exp_t[:, kt, qt * 128:(qt + 1) * 128],
                                v_aug[kt][:, h, :],
                                start=(kt == 0), stop=(kt == KT - 1))
                    rec = rpool.tile([128, 4, 1], F32, tag="rec",
                                     name=f"rec{qb}_{h}")
                    nc.vector.reciprocal(rec[:], o_ps[:, :, HD:HD + 1])
                    nc.vector.tensor_tensor(
                        o_sb[:, :, h * HD:(h + 1) * HD], o_ps[:, :, 0:HD],
                        rec.broadcast_to((128, 4, HD)), ALU.mult)
                    # PE filler while ScalarE chews on exp tiles
                    if h == 1 and qb + 2 < NQB:
                        qproj(qb + 2)

                # transpose normalized o back to [od, q] (pairs of heads)
                for c in range(2):
                    t_ps = ps.tile([128, QB], BF16, tag="tp", bufs=1,
                                   name=f"tp{qb}_{c}")
                    for qt in range(4):
                        nc.tensor.transpose(
                            t_ps[:, qt * 128:(qt + 1) * 128],
                            o_sb[:, qt, c * 128:(c + 1) * 128], id_sb[:])
                    nc.vector.tensor_copy(o_all[:, c, qb * QB:(qb + 1) * QB],
                                          t_ps[:])

                # output projection partial for this query block
                for dt in range(NCH):
                    yp = ps.tile([128, QB], F32, tag="proj", name=f"yp{qb}_{dt}")
                    for c in range(2):
                        nc.tensor.matmul(
                            yp[:], wo_sb[:, c, dt * 128:(dt + 1) * 128],
                            o_all[:, c, qb * QB:(qb + 1) * QB],
                            start=(c == 0), stop=(c == 1))
                    y_sb = ypool.tile([128, QB], F32, tag="y",
                                      name=f"y{qb}_{dt}")
                    if dt % 2 == 0:
                        nc.vector.tensor_copy(y_sb[:], yp[:])
                    else:
                        nc.scalar.copy(y_sb[:], yp[:])
                    eng = nc.sync if dt % 2 == 0 else nc.gpsimd
                    eng.dma_start(y_d[:, dt, qb * QB:(qb + 1) * QB], y_sb[:])

    nc.compile()
    _CACHE["nc"] = nc
    return nc


def _to_pko(a2d, dtype=ml_dtypes.bfloat16):
    """(D_in, M) row-major -> [128, D_in//128, M] with d = ko*128 + p."""
    d_in, m = a2d.shape
    return np.ascontiguousarray(
        a2d.reshape(d_in // 128, 128, m).transpose(1, 0, 2)).astype(dtype)


def kernel(x, condition, end_inds, in_proj_w, in_proj_b, out_w, out_b):
    nc = _build()

    x = np.asarray(x, dtype=np.float32)
    condition = np.asarray(condition, dtype=np.float32)
    end_inds = np.asarray(end_inds, dtype=np.int32)
    in_proj_w = np.asarray(in_proj_w, dtype=np.float32)
    in_proj_b = np.asarray(in_proj_b, dtype=np.float32)
    out_w = np.asarray(out_w, dtype=np.float32)
    out_b = np.asarray(out_b, dtype=np.float32)

    ident = np.eye(128, dtype=ml_dtypes.bfloat16)
    wo_full = np.ascontiguousarray(out_w.T)          # (od, ydim)

    in_maps = []
    per_core = []
    for core in range(8):
        b, hq = divmod(core, 4)
        inp = np.concatenate([x[b], condition[b]], axis=0)       # (3072, 1024)
        e = int(end_inds[b])
        sel = np.concatenate([inp[e - W:e], inp[T_IN + e - W:T_IN + e]], axis=0)
        lo = hq * ODC
        wq = 0.125 * in_proj_w[lo:lo + ODC]                      # (256, 1024)
        wk = in_proj_w[D + lo:D + lo + ODC]
        wv = in_proj_w[2 * D + lo:2 * D + lo + ODC]
        bq = np.ascontiguousarray(
            (0.125 * in_proj_b[lo:lo + ODC]).reshape(2, 128).T).astype(np.float32)
        bk = np.ascontiguousarray(
            in_proj_b[D + lo:D + lo + ODC].reshape(2, 128).T).astype(np.float32)
        in_maps.append({
            "x": _to_pko(np.ascontiguousarray(inp.T)),
            "xkv": _to_pko(np.ascontiguousarray(sel.T)),
            "wq": _to_pko(np.ascontiguousarray(wq.T)),
            "wk": _to_pko(np.ascontiguousarray(wk.T)),
            "wv": _to_pko(np.ascontiguousarray(wv.T)),
            "wo": _to_pko(np.ascontiguousarray(wo_full[lo:lo + ODC])),
            "bq": bq, "bk": bk, "ident": ident,
        })
        per_core.append((b, hq))

    res = run_bass_kernel_spmd(nc, in_maps, core_ids=list(range(8)))

    out = np.zeros((B, S, D), dtype=np.float32)
    for core in range(8):
        b, hq = per_core[core]
        yv = np.asarray(res.results[core]["y"], dtype=np.float32)  # [128, 8, 3072]
        out[b] += yv.transpose(2, 1, 0).reshape(S, D)
    bo_eff = out_b + out_w @ in_proj_b[2 * D:3 * D]
    out += bo_eff.astype(np.float32)
    return out
